# revision 3
# baseline (speedup 1.0000x reference)
"""Trainium2 Bass kernel for nn_EquivariantProductBasisBlock (c-major design).

Computation (per node n, channel c):
  s = nf[n,c,0]; v = nf[n,c,1:4]; v2 = |v|^2
  out0 = w0*s + w1*s^2 + w2'*v2 + w3*s^3 + w4*s*v2      (w_p = W0[sp[n],p,c], w2' scaled)
  B1   = u0 + u1'*s + u2'*s^2 + u3'*v2                  (u_p = W1[sp[n],p,c], scaled)
  out1m = B1 * v_m
  y0 = out0 @ L0 / sqrt(C);  y1m = out1m @ L1 / sqrt(C)
  y[n,c,:] = [y0, y1x, y1y, y1z] + sc[n,c,:]

Layout strategy: channel-major ([c on partitions, nodes on the free axis]),
with nodes SORTED BY SPECIES on the host so the per-node path weights become
per-partition [128,1] scalar columns (tensor_scalar runs in 4x DVE mode).
This eliminates the one-hot gather matmuls and all transposes.

  - host: per-core node permutation with identical per-species segment
    lengths on every core (SPMD: one program), fp16 plane-major chunk-blocked
    nf/sc/out arrays
  - DVE: polynomial streams via tensor_scalar (4x) / tensor_tensor (2x)
  - ACT: v squares + PSUM->SBUF fp16 copies
  - PE:  channel mixing only — 5 streams (X4, YV -> y0; B1*v_m -> y1m)
  - sc added by fp16 CCE accumulate-DMA (SWDGE), out stored fp16,
    upcast + un-permuted on the host
"""

import numpy as np

N_CORES = 8
N_NODES = 65536
C = 128
E = 10
CH = 2048          # nodes per DVE/ACT chunk
SUB = 512          # nodes per PSUM sub-chunk (one bank per output plane)

INV_SQ3 = 1.0 / np.sqrt(3.0)
SQ2 = float(np.sqrt(2.0))
SQ3 = float(np.sqrt(3.0))
SQ35 = float(np.sqrt(3.0 / 5.0))

_CACHE = {}


# ---------------------------------------------------------------------------
# Workarounds for the walrus build in this container: it rejects any
# instruction carrying more than one sync-wait ("Too many sync wait
# commands").  Split extra waits onto same-engine NOPs preceding the
# instruction (identical semantics: the engine queue is FIFO).
# ---------------------------------------------------------------------------
def _apply_patches():
    import concourse.tile as tile
    from concourse import mybir
    from concourse.vector_clock import ScopedClock

    if getattr(tile.TileContext, "_singlewait_patched", False):
        return

    def _patched_drain_and_barrier(self, tick_clock, wait_clock):
        nc = self.nc
        probe = nc.sync.nop()
        wait_clock.add_sem_waits(probe.ins, ScopedClock({None: tick_clock.global_clock}))
        si = probe.ins.sync_info
        waits = list(si.on_wait) if si and si.on_wait else []
        if len(waits) > 1:
            probe.ins.sync_info = type(si)(on_wait=waits[:1], on_update=[])
            for w in waits[1:]:
                extra = nc.sync.nop()
                extra.ins.sync_info = type(si)(on_wait=[w], on_update=[])
        nc.sync.drain()
        nc.all_engine_barrier()
        assert self.sems is not None
        popped = nc._tile_sem_poison_stack.pop()
        assert popped is self._sem_poison
        nc.clear_and_free_semaphores(list(self.sems.allocated().values()))
        nc.all_engine_barrier()

    _orig_commit = tile.TileContext._commit_instruction

    def _split_commit(self, inst, lazy_reg_writes=True):
        si = getattr(inst, "sync_info", None)
        if (si is not None and si.on_wait and len(si.on_wait) > 1
                and getattr(inst, "engine", mybir.EngineType.Unassigned)
                != mybir.EngineType.Unassigned):
            waits = list(si.on_wait)
            for w in waits[:-1]:
                nop = mybir.InstNoOp(name=self.nc.get_next_instruction_name(),
                                     ins=[], outs=[], engine=inst.engine)
                nop.sync_info = mybir.SyncInfo(on_wait=[w], on_update=[])
                _orig_commit(self, nop, lazy_reg_writes=False)
            inst.sync_info = mybir.SyncInfo(on_wait=[waits[-1]],
                                            on_update=list(si.on_update or []))
        return _orig_commit(self, inst, lazy_reg_writes)

    tile.TileContext._drain_and_barrier = _patched_drain_and_barrier
    tile.TileContext._commit_instruction = _split_commit
    tile.TileContext._singlewait_patched = True


def _chunk_sizes(n_pad):
    sizes = []
    left = n_pad
    while left > 0:
        sizes.append(min(CH, left))
        left -= min(CH, left)
    return sizes


def _build_program(seg_bounds, n_pad, reps=1):
    """seg_bounds: list of (start, end, species) node ranges, sorted,
    covering [0, n_pad) — identical on every core."""
    import concourse.bass as bass
    import concourse.tile as tile
    from concourse import mybir
    from contextlib import ExitStack

    _apply_patches()
    F16 = mybir.dt.float16
    F32 = mybir.dt.float32
    nc = bass.Bass()

    nf_d = nc.declare_dram_parameter("nf", [128, 4 * n_pad], F16, isOutput=False)
    sc_d = nc.declare_dram_parameter("sc", [128, 4 * n_pad], F16, isOutput=False)
    w_d = nc.declare_dram_parameter("wc", [128, 9 * E], F32, isOutput=False)
    l0_d = nc.declare_dram_parameter("l0", [C, C], F16, isOutput=False)
    l1_d = nc.declare_dram_parameter("l1", [C, C], F16, isOutput=False)
    out_d = nc.declare_dram_parameter("out", [128, 4 * n_pad], F16, isOutput=True)

    mult = mybir.AluOpType.mult
    add = mybir.AluOpType.add
    Square = mybir.ActivationFunctionType.Square

    sizes = _chunk_sizes(n_pad)

    with tile.TileContext(nc) as tc, ExitStack() as ctx:
        consts = ctx.enter_context(tc.tile_pool(name="consts", bufs=1))
        work = ctx.enter_context(tc.tile_pool(name="work", bufs=2))
        ps = ctx.enter_context(tc.tile_pool(name="ps", bufs=2, space="PSUM"))

        t_w = consts.tile([128, 9 * E], F32)
        nc.sync.dma_start(out=t_w, in_=w_d[:, :])
        t_l0 = consts.tile([C, C], F16)
        nc.sync.dma_start(out=t_l0, in_=l0_d[:, :])
        t_l1 = consts.tile([C, C], F16)
        nc.sync.dma_start(out=t_l1, in_=l1_d[:, :])

        def wcol(e, j):
            return t_w[:, 9 * e + j:9 * e + j + 1]

        chunks = []
        base = 0
        for sz in sizes:
            chunks.append((base, sz))
            base += sz

        for base, sz in [c for _ in range(reps) for c in chunks]:
            # --- load nf chunk: [128, 4, sz] planes s|vx|vy|vz ---
            t_nf = work.tile([128, 4, sz], F16, tag="nf")
            nc.sync.dma_start(out=t_nf, in_=nf_d[:, 4 * base:4 * (base + sz)])
            s = t_nf[:, 0, :]
            vpl = t_nf[:, 1:4, :]

            # --- squares on ACT, v2 adds on DVE ---
            t_vv = work.tile([128, 3, sz], F16, tag="vv")
            nc.scalar.activation(out=t_vv, in_=vpl, func=Square)
            t_v2 = work.tile([128, sz], F16, tag="v2")
            nc.vector.tensor_tensor(out=t_v2, in0=t_vv[:, 0, :],
                                    in1=t_vv[:, 1, :], op=add)
            nc.vector.tensor_tensor(out=t_v2, in0=t_v2,
                                    in1=t_vv[:, 2, :], op=add)

            # --- per-species-segment tensor_scalar ops (4x mode) ---
            t_x1 = work.tile([128, sz], F16, tag="x1")
            t_y1 = work.tile([128, sz], F16, tag="y1")
            t_u1 = work.tile([128, sz], F16, tag="u1")
            t_u3 = work.tile([128, sz], F16, tag="u3")
            segs = [(max(base, lo) - base, min(base + sz, hi) - base, e)
                    for lo, hi, e in seg_bounds
                    if lo < base + sz and hi > base]
            for lo, hi, e in segs:
                sl = slice(lo, hi)
                nc.vector.tensor_scalar(out=t_x1[:, sl], in0=s[:, sl],
                                        scalar1=wcol(e, 3), scalar2=wcol(e, 1),
                                        op0=mult, op1=add)
                nc.vector.tensor_scalar(out=t_y1[:, sl], in0=s[:, sl],
                                        scalar1=wcol(e, 4), scalar2=wcol(e, 2),
                                        op0=mult, op1=add)
                nc.vector.tensor_scalar(out=t_u1[:, sl], in0=s[:, sl],
                                        scalar1=wcol(e, 7), scalar2=wcol(e, 6),
                                        op0=mult, op1=add)
                nc.vector.tensor_scalar(out=t_u3[:, sl], in0=t_v2[:, sl],
                                        scalar1=wcol(e, 8), scalar2=wcol(e, 5),
                                        op0=mult, op1=add)

            # --- chunk-wide tensor_tensor chains (2x mode) ---
            t_x2 = work.tile([128, sz], F16, tag="x2")
            nc.vector.tensor_tensor(out=t_x2, in0=t_x1, in1=s, op=mult)
            t_yv = work.tile([128, sz], F16, tag="yv")
            nc.vector.tensor_tensor(out=t_yv, in0=t_y1, in1=t_v2, op=mult)
            t_u2 = work.tile([128, sz], F16, tag="u2")
            nc.vector.tensor_tensor(out=t_u2, in0=t_u1, in1=s, op=mult)
            t_x3 = work.tile([128, sz], F16, tag="x3")
            for lo, hi, e in segs:
                nc.vector.tensor_scalar(out=t_x3[:, lo:hi], in0=t_x2[:, lo:hi],
                                        scalar1=wcol(e, 0), scalar2=None,
                                        op0=add)
            t_x4 = work.tile([128, sz], F16, tag="x4")
            nc.vector.tensor_tensor(out=t_x4, in0=t_x3, in1=s, op=mult)
            t_b1 = work.tile([128, sz], F16, tag="b1")
            nc.vector.tensor_tensor(out=t_b1, in0=t_u2, in1=t_u3, op=add)
            # o1 = B1 (bcast over planes) * v  — overwrite vv (dead)
            t_o1 = t_vv
            b1_bc = bass.AP(tensor=t_b1.tensor, offset=t_b1.offset,
                            ap=[t_b1.ap[0], [0, 3], [1, sz]])
            nc.vector.tensor_tensor(out=t_o1, in0=b1_bc, in1=vpl, op=mult)

            # --- PE channel mixing + ACT copy per PSUM sub-chunk ---
            t_y = work.tile([128, 4, sz], F16, tag="y")
            nsub = (sz + SUB - 1) // SUB
            for k in range(nsub):
                c0 = k * SUB
                c1 = min(sz, c0 + SUB)
                w_ = c1 - c0
                p = ps.tile([128, 4 * w_], F32, tag="py")
                nc.tensor.matmul(p[:, 0:w_], lhsT=t_l0, rhs=t_x4[:, c0:c1],
                                 start=True, stop=False)
                nc.tensor.matmul(p[:, 0:w_], lhsT=t_l0, rhs=t_yv[:, c0:c1],
                                 start=False, stop=True)
                for m in range(3):
                    nc.tensor.matmul(p[:, (m + 1) * w_:(m + 2) * w_],
                                     lhsT=t_l1, rhs=t_o1[:, m, c0:c1],
                                     start=True, stop=True)
                # psum f32 -> y fp16 (ACT), strided into [128, 4, sz]
                out_ap = bass.AP(tensor=t_y.tensor,
                                 offset=t_y.offset + c0,
                                 ap=[t_y.ap[0], [sz, 4], [1, w_]])
                in_ap = bass.AP(tensor=p.tensor, offset=p.offset,
                                ap=[p.ap[0], [w_, 4], [1, w_]])
                nc.scalar.copy(out=out_ap, in_=in_ap)

            # --- sc accumulate (fp16 CCE; <=2KB per partition per instr,
            # larger accum DMAs silently corrupt / wedge the device) ---
            t_y_flat = bass.AP(tensor=t_y.tensor, offset=t_y.offset,
                               ap=[t_y.ap[0], [1, 4 * sz]])
            for j in range(0, 4 * sz, 1024):
                w_j = min(1024, 4 * sz - j)
                nc.gpsimd.dma_start(
                    out=bass.AP(tensor=t_y.tensor, offset=t_y.offset + j,
                                ap=[t_y.ap[0], [1, w_j]]),
                    in_=sc_d[:, 4 * base + j:4 * base + j + w_j],
                    accum_op=add)
            nc.sync.dma_start(out=out_d[:, 4 * base:4 * (base + sz)], in_=t_y)

    return nc


def _prep_host(inputs):
    """Returns (in_maps, seg_bounds, n_pad, perms)."""
    nf = np.asarray(inputs["node_feats"], dtype=np.float32)
    sc = np.asarray(inputs["sc"], dtype=np.float32)
    sp = np.asarray(inputs["node_species"]).astype(np.int64)
    W0 = np.asarray(inputs["W0"], dtype=np.float32)
    W1 = np.asarray(inputs["W1"], dtype=np.float32)
    L0 = np.asarray(inputs["L0"], dtype=np.float32)
    L1 = np.asarray(inputs["L1"], dtype=np.float32)

    # --- per-species weight columns [128, 9E] f32 ---
    wc = np.empty((C, 9 * E), np.float32)
    for e in range(E):
        wc[:, 9 * e + 0] = W0[e, 0]
        wc[:, 9 * e + 1] = W0[e, 1]
        wc[:, 9 * e + 2] = W0[e, 2] * INV_SQ3
        wc[:, 9 * e + 3] = W0[e, 3]
        wc[:, 9 * e + 4] = W0[e, 4]
        wc[:, 9 * e + 5] = W1[e, 0]
        wc[:, 9 * e + 6] = W1[e, 1] * SQ2
        wc[:, 9 * e + 7] = W1[e, 2] * SQ3
        wc[:, 9 * e + 8] = W1[e, 3] * SQ35
    inv_sqrt_c = np.float32(1.0 / np.sqrt(C))
    l0 = np.ascontiguousarray((L0 * inv_sqrt_c).astype(np.float16))
    l1 = np.ascontiguousarray((L1 * inv_sqrt_c).astype(np.float16))

    # --- species sort; equal per-core per-species segment lengths ---
    counts = np.bincount(sp, minlength=E)
    order = np.argsort(sp, kind="stable")
    starts = np.concatenate([[0], np.cumsum(counts)])
    q = [(int(counts[e]) + N_CORES - 1) // N_CORES for e in range(E)]
    n0 = sum(q)
    n_pad = ((n0 + 127) // 128) * 128
    q[E - 1] += n_pad - n0

    seg_bounds = []
    pos = 0
    for e in range(E):
        seg_bounds.append((pos, pos + q[e], e))
        pos += q[e]

    perms = []
    for k in range(N_CORES):
        parts = []
        for e in range(E):
            base_q = (int(counts[e]) + N_CORES - 1) // N_CORES
            lo = starts[e] + k * base_q
            hi = min(starts[e] + int(counts[e]), lo + base_q)
            seg = order[lo:hi] if hi > lo else order[starts[e]:starts[e] + 1]
            if len(seg) < q[e]:
                seg = np.concatenate([seg, np.repeat(seg[-1], q[e] - len(seg))])
            parts.append(seg)
        perms.append(np.concatenate(parts))

    sizes = _chunk_sizes(n_pad)
    in_maps = []
    for k in range(N_CORES):
        P = perms[k]
        # [n_pad, 128, 4] -> [4, 128, n_pad] planes (s, vx, vy, vz)
        nf_k = nf[P].transpose(2, 1, 0).astype(np.float16)
        sc_k = sc[P].transpose(2, 1, 0).astype(np.float16)
        nf_dev = np.empty((128, 4 * n_pad), np.float16)
        sc_dev = np.empty((128, 4 * n_pad), np.float16)
        b = 0
        for szc in sizes:
            blk = nf_k[:, :, b:b + szc].transpose(1, 0, 2).reshape(128, 4 * szc)
            nf_dev[:, 4 * b:4 * (b + szc)] = blk
            blk = sc_k[:, :, b:b + szc].transpose(1, 0, 2).reshape(128, 4 * szc)
            sc_dev[:, 4 * b:4 * (b + szc)] = blk
            b += szc
        in_maps.append({"nf": nf_dev, "sc": sc_dev, "wc": wc,
                        "l0": l0, "l1": l1})
    return in_maps, seg_bounds, n_pad, perms


def _gather_output(res, n_pad, perms):
    sizes = _chunk_sizes(n_pad)
    y_full = np.empty((N_NODES, C, 4), np.float32)
    for k in range(N_CORES):
        o = res.results[k]["out"]
        y_sorted = np.empty((128, 4, n_pad), np.float32)
        b = 0
        for szc in sizes:
            blk = o[:, 4 * b:4 * (b + szc)].astype(np.float32)
            y_sorted[:, :, b:b + szc] = blk.reshape(128, 4, szc)
            b += szc
        # [d, plane, n] -> [n, d, plane]
        y_full[perms[k]] = y_sorted.transpose(2, 0, 1)
    return y_full


def kernel(**inputs):
    from concourse.bass_utils import run_bass_kernel_spmd

    in_maps, seg_bounds, n_pad, perms = _prep_host(inputs)
    key = (tuple(seg_bounds), n_pad)
    if _CACHE.get("key") != key:
        _CACHE["nc"] = _build_program(seg_bounds, n_pad)
        _CACHE["key"] = key
    nc = _CACHE["nc"]

    res = run_bass_kernel_spmd(nc, in_maps, core_ids=list(range(N_CORES)),
                               **_CACHE.get("run_kwargs", {}))
    _CACHE["last_result"] = res
    return _gather_output(res, n_pad, perms)


# revision 6
# speedup vs baseline: 2.9274x; 2.9274x over previous
"""Trainium2 Bass kernel for nn_EquivariantProductBasisBlock (c-major design).

Computation (per node n, channel c):
  s = nf[n,c,0]; v = nf[n,c,1:4]; v2 = |v|^2
  out0 = w0*s + w1*s^2 + w2'*v2 + w3*s^3 + w4*s*v2      (w_p = W0[sp[n],p,c])
  B1   = u0 + u1'*s + u2'*s^2 + u3'*v2                  (u_p = W1[sp[n],p,c])
  out1m = B1 * v_m
  y0 = out0 @ L0 / sqrt(C);  y1m = out1m @ L1 / sqrt(C)
  y[n,c,:] = [y0, y1x, y1y, y1z] + sc[n,c,:]

Layout: channel-major (c on partitions, nodes on the free axis), nodes
SORTED BY SPECIES on the host so per-node path weights become per-partition
[128,1] scalar columns (tensor_scalar in 4x DVE mode; fused custom DVE ops
for the cubic Horner chain and the B1 assembly).  No gather matmuls, no
transposes.

  - host: per-core node permutation with identical per-species segment
    lengths on every core (SPMD: one program); fp16 plane-major
    chunk-blocked nf/sc/out arrays
  - DVE: polynomial streams; ACT: v squares + PSUM->SBUF fp16 copies
  - PE:  channel mixing (X4, YV -> y0; B1*v_m -> y1m) plus sc added via
    identity matmuls accumulating into the same PSUM banks (the fp16
    CCE accumulate-DMA path measured ~26us/pass slower)
  - loads on the SP HWDGE queue, stores on the Activation HWDGE queue
    (a single queue serializes loads behind stores)
"""

import numpy as np

N_CORES = 8
N_NODES = 65536
C = 128
E = 10
CH = 1024          # nodes per DVE/ACT chunk
SUB = 512          # nodes per PSUM sub-chunk (one bank per output plane)

INV_SQ3 = 1.0 / np.sqrt(3.0)
SQ2 = float(np.sqrt(2.0))
SQ3 = float(np.sqrt(3.0))
SQ35 = float(np.sqrt(3.0 / 5.0))

_CACHE = {}


# ---------------------------------------------------------------------------
# Workarounds for the walrus build in this container: it rejects any
# instruction carrying more than one sync-wait ("Too many sync wait
# commands").  Split extra waits onto same-engine NOPs preceding the
# instruction (identical semantics: the engine queue is FIFO).
# ---------------------------------------------------------------------------
def _apply_patches():
    import concourse.tile as tile
    from concourse import mybir
    from concourse.vector_clock import ScopedClock

    if getattr(tile.TileContext, "_singlewait_patched", False):
        return

    def _patched_drain_and_barrier(self, tick_clock, wait_clock):
        nc = self.nc
        probe = nc.sync.nop()
        wait_clock.add_sem_waits(probe.ins, ScopedClock({None: tick_clock.global_clock}))
        si = probe.ins.sync_info
        waits = list(si.on_wait) if si and si.on_wait else []
        if len(waits) > 1:
            probe.ins.sync_info = type(si)(on_wait=waits[:1], on_update=[])
            for w in waits[1:]:
                extra = nc.sync.nop()
                extra.ins.sync_info = type(si)(on_wait=[w], on_update=[])
        nc.sync.drain()
        nc.all_engine_barrier()
        assert self.sems is not None
        popped = nc._tile_sem_poison_stack.pop()
        assert popped is self._sem_poison
        nc.clear_and_free_semaphores(list(self.sems.allocated().values()))
        nc.all_engine_barrier()

    _orig_commit = tile.TileContext._commit_instruction

    def _split_commit(self, inst, lazy_reg_writes=True):
        si = getattr(inst, "sync_info", None)
        if (si is not None and si.on_wait and len(si.on_wait) > 1
                and getattr(inst, "engine", mybir.EngineType.Unassigned)
                != mybir.EngineType.Unassigned):
            waits = list(si.on_wait)
            for w in waits[:-1]:
                nop = mybir.InstNoOp(name=self.nc.get_next_instruction_name(),
                                     ins=[], outs=[], engine=inst.engine)
                nop.sync_info = mybir.SyncInfo(on_wait=[w], on_update=[])
                _orig_commit(self, nop, lazy_reg_writes=False)
            inst.sync_info = mybir.SyncInfo(on_wait=[waits[-1]],
                                            on_update=list(si.on_update or []))
        return _orig_commit(self, inst, lazy_reg_writes)

    tile.TileContext._drain_and_barrier = _patched_drain_and_barrier
    tile.TileContext._commit_instruction = _split_commit
    tile.TileContext._singlewait_patched = True


# ---------------------------------------------------------------------------
# Custom fused DVE ops (registered at runtime; shas computed in-process).
#   POLY3_HORNER_ANT:       out = ((in0*s0 + s1)*in0 + in1[0])*in0
#                           (C3 spilled to Src1: pass the third scalar via
#                            in1 as a [P,1] AP)
#   AFFINE_MUL_THEN_ADD_ANT: out = (in0*s0 + s1)*in0 + in1
# ---------------------------------------------------------------------------
def _get_custom_ops():
    if "ops" in _CACHE:
        return _CACHE["ops"]
    from concourse.dve_spec import (Spec, Src0, Src1, C0, C1, C3, lower,
                                    _spill_c3_to_src1)
    from concourse.dve_uop import DveOpSpec
    from concourse import dve_ops as DO

    def register(name, spec):
        for op in DO.OPS:
            if op.name == name:
                return op
        row = DO._CUSTOM_DVE_ROW_BASE + len(DO.OPS)
        assert row < 0x20
        DO._SUB_OPCODE_FOR_NAME[name] = row
        shas = {}
        for v in ("v3", "v4"):
            uops = lower(spec, ver=v)
            shas[v] = DveOpSpec(name=name, opcode=row, uops=uops,
                                rd1_en=DO.has_src1(spec)).sha(v)
        op = DO.DveOp(name, spec, subdim=False, uops_sha=shas)
        DO.OPS.append(op)
        DO.CUSTOM_DVE_SPECS[name] = spec
        return op

    bodyA = (((Src0 * C0 + C1) * Src0) + C3) * Src0
    opA = register("POLY3_HORNER_ANT", Spec(
        body=_spill_c3_to_src1(bodyA),
        reference=lambda in0, in1, s0, s1, imm2:
            (((in0.astype(np.float32) * s0 + s1) * in0) + in1) * in0))
    opU = register("AFFINE_MUL_THEN_ADD_ANT", Spec(
        body=((Src0 * C0 + C1) * Src0) + Src1,
        reference=lambda in0, in1, s0, s1, imm2:
            ((in0.astype(np.float32) * s0 + s1) * in0) + in1))
    _CACHE["ops"] = (opA, opU)
    return _CACHE["ops"]


def _chunk_sizes(n_pad):
    sizes = []
    left = n_pad
    while left > 0:
        sizes.append(min(CH, left))
        left -= min(CH, left)
    return sizes


def _build_program(seg_bounds, n_pad, reps=1):
    """seg_bounds: list of (start, end, species) node ranges, sorted,
    covering [0, n_pad) — identical on every core."""
    import concourse.bass as bass
    import concourse.tile as tile
    from concourse import mybir
    from contextlib import ExitStack

    _apply_patches()
    F16 = mybir.dt.float16
    F32 = mybir.dt.float32
    nc = bass.Bass()

    nf_d = nc.declare_dram_parameter("nf", [128, 4 * n_pad], F16, isOutput=False)
    sc_d = nc.declare_dram_parameter("sc", [128, 4 * n_pad], F16, isOutput=False)
    w_d = nc.declare_dram_parameter("wc", [128, 9 * E], F32, isOutput=False)
    l0_d = nc.declare_dram_parameter("l0", [C, C], F16, isOutput=False)
    l1_d = nc.declare_dram_parameter("l1", [C, C], F16, isOutput=False)
    id_d = nc.declare_dram_parameter("idm", [C, C], F16, isOutput=False)
    out_d = nc.declare_dram_parameter("out", [128, 4 * n_pad], F16, isOutput=True)

    mult = mybir.AluOpType.mult
    add = mybir.AluOpType.add
    Square = mybir.ActivationFunctionType.Square

    sizes = _chunk_sizes(n_pad)

    with tile.TileContext(nc) as tc, ExitStack() as ctx:
        consts = ctx.enter_context(tc.tile_pool(name="consts", bufs=1))
        work = ctx.enter_context(tc.tile_pool(name="work", bufs=2))
        ps = ctx.enter_context(tc.tile_pool(name="ps", bufs=2, space="PSUM"))

        t_w = consts.tile([128, 9 * E], F32)
        nc.sync.dma_start(out=t_w, in_=w_d[:, :])
        t_l0 = consts.tile([C, C], F16)
        nc.sync.dma_start(out=t_l0, in_=l0_d[:, :])
        t_l1 = consts.tile([C, C], F16)
        nc.sync.dma_start(out=t_l1, in_=l1_d[:, :])
        t_id = consts.tile([C, C], F16)
        nc.sync.dma_start(out=t_id, in_=id_d[:, :])

        def wcol(e, j):
            return t_w[:, 9 * e + j:9 * e + j + 1]

        chunks = []
        base = 0
        for szc in sizes:
            chunks.append((base, szc))
            base += szc

        for base, sz in [c for _ in range(reps) for c in chunks]:
            # --- loads (SP queue): nf and sc chunk [128, 4, sz] ---
            t_nf = work.tile([128, 4, sz], F16, tag="nf")
            nc.sync.dma_start(out=t_nf, in_=nf_d[:, 4 * base:4 * (base + sz)])
            t_sc = work.tile([128, 4, sz], F16, tag="sc")
            nc.sync.dma_start(out=t_sc, in_=sc_d[:, 4 * base:4 * (base + sz)])
            s = t_nf[:, 0, :]
            vpl = t_nf[:, 1:4, :]

            # --- squares on ACT, v2 adds on DVE ---
            t_vv = work.tile([128, 3, sz], F16, tag="vv")
            nc.scalar.activation(out=t_vv, in_=vpl, func=Square)
            t_v2 = work.tile([128, sz], F16, tag="v2")
            nc.vector.tensor_tensor(out=t_v2, in0=t_vv[:, 0, :],
                                    in1=t_vv[:, 1, :], op=add)
            nc.vector.tensor_tensor(out=t_v2, in0=t_v2,
                                    in1=t_vv[:, 2, :], op=add)

            # --- per-species-segment ops ---
            segs = [(max(base, lo) - base, min(base + sz, hi) - base, e)
                    for lo, hi, e in seg_bounds
                    if lo < base + sz and hi > base]
            t_x1 = work.tile([128, sz], F16, tag="x1")
            t_y1 = work.tile([128, sz], F16, tag="y1")
            t_u1 = work.tile([128, sz], F16, tag="u1")
            t_u3 = work.tile([128, sz], F16, tag="u3")
            for lo, hi, e in segs:
                sl = slice(lo, hi)
                nc.vector.tensor_scalar(out=t_x1[:, sl], in0=s[:, sl],
                                        scalar1=wcol(e, 3), scalar2=wcol(e, 1),
                                        op0=mult, op1=add)
                nc.vector.tensor_scalar(out=t_y1[:, sl], in0=s[:, sl],
                                        scalar1=wcol(e, 4), scalar2=wcol(e, 2),
                                        op0=mult, op1=add)
                nc.vector.tensor_scalar(out=t_u1[:, sl], in0=s[:, sl],
                                        scalar1=wcol(e, 7), scalar2=wcol(e, 6),
                                        op0=mult, op1=add)
                nc.vector.tensor_scalar(out=t_u3[:, sl], in0=t_v2[:, sl],
                                        scalar1=wcol(e, 8), scalar2=wcol(e, 5),
                                        op0=mult, op1=add)
            # --- chunk-wide streams ---
            t_x2 = work.tile([128, sz], F16, tag="x2")
            nc.vector.tensor_tensor(out=t_x2, in0=t_x1, in1=s, op=mult)
            t_x3 = work.tile([128, sz], F16, tag="x3")
            for lo, hi, e in segs:
                nc.vector.tensor_scalar(out=t_x3[:, lo:hi], in0=t_x2[:, lo:hi],
                                        scalar1=wcol(e, 0), scalar2=None,
                                        op0=add)
            t_x4 = work.tile([128, sz], F16, tag="x4")
            nc.vector.tensor_tensor(out=t_x4, in0=t_x3, in1=s, op=mult)
            t_u2 = work.tile([128, sz], F16, tag="u2")
            nc.vector.tensor_tensor(out=t_u2, in0=t_u1, in1=s, op=mult)
            t_b1 = work.tile([128, sz], F16, tag="b1")
            nc.vector.tensor_tensor(out=t_b1, in0=t_u2, in1=t_u3, op=add)
            t_yv = work.tile([128, sz], F16, tag="yv")
            nc.vector.tensor_tensor(out=t_yv, in0=t_y1, in1=t_v2, op=mult)
            t_o1 = t_vv  # vv is dead after v2
            b1_bc = bass.AP(tensor=t_b1.tensor, offset=t_b1.offset,
                            ap=[t_b1.ap[0], [0, 3], [1, sz]])
            nc.vector.tensor_tensor(out=t_o1, in0=b1_bc, in1=vpl, op=mult)

            # --- PE channel mixing (+ sc via identity matmuls) + ACT copy ---
            t_y = work.tile([128, 4, sz], F16, tag="y")
            nsub = (sz + SUB - 1) // SUB
            for k in range(nsub):
                c0 = k * SUB
                c1 = min(sz, c0 + SUB)
                w_ = c1 - c0
                p = ps.tile([128, 4 * w_], F32, tag="py")
                nc.tensor.matmul(p[:, 0:w_], lhsT=t_l0, rhs=t_x4[:, c0:c1],
                                 start=True, stop=False)
                nc.tensor.matmul(p[:, 0:w_], lhsT=t_l0, rhs=t_yv[:, c0:c1],
                                 start=False, stop=False)
                nc.tensor.matmul(p[:, 0:w_], lhsT=t_id, rhs=t_sc[:, 0, c0:c1],
                                 start=False, stop=True)
                for m in range(3):
                    nc.tensor.matmul(p[:, (m + 1) * w_:(m + 2) * w_],
                                     lhsT=t_l1, rhs=t_o1[:, m, c0:c1],
                                     start=True, stop=False)
                for m in range(3):
                    nc.tensor.matmul(p[:, (m + 1) * w_:(m + 2) * w_],
                                     lhsT=t_id, rhs=t_sc[:, m + 1, c0:c1],
                                     start=False, stop=True)
                out_ap = bass.AP(tensor=t_y.tensor,
                                 offset=t_y.offset + c0,
                                 ap=[t_y.ap[0], [sz, 4], [1, w_]])
                in_ap = bass.AP(tensor=p.tensor, offset=p.offset,
                                ap=[p.ap[0], [w_, 4], [1, w_]])
                nc.scalar.copy(out=out_ap, in_=in_ap)

            # --- store on the Activation HWDGE queue ---
            nc.scalar.dma_start(out=out_d[:, 4 * base:4 * (base + sz)], in_=t_y)

    return nc


def _prep_host(inputs):
    """Returns (in_maps, seg_bounds, n_pad, perms)."""
    nf = np.asarray(inputs["node_feats"], dtype=np.float32)
    sc = np.asarray(inputs["sc"], dtype=np.float32)
    sp = np.asarray(inputs["node_species"]).astype(np.int64)
    W0 = np.asarray(inputs["W0"], dtype=np.float32)
    W1 = np.asarray(inputs["W1"], dtype=np.float32)
    L0 = np.asarray(inputs["L0"], dtype=np.float32)
    L1 = np.asarray(inputs["L1"], dtype=np.float32)

    # --- per-species weight columns [128, 9E] f32 ---
    wc = np.empty((C, 9 * E), np.float32)
    for e in range(E):
        wc[:, 9 * e + 0] = W0[e, 0]
        wc[:, 9 * e + 1] = W0[e, 1]
        wc[:, 9 * e + 2] = W0[e, 2] * INV_SQ3
        wc[:, 9 * e + 3] = W0[e, 3]
        wc[:, 9 * e + 4] = W0[e, 4]
        wc[:, 9 * e + 5] = W1[e, 0]
        wc[:, 9 * e + 6] = W1[e, 1] * SQ2
        wc[:, 9 * e + 7] = W1[e, 2] * SQ3
        wc[:, 9 * e + 8] = W1[e, 3] * SQ35
    inv_sqrt_c = np.float32(1.0 / np.sqrt(C))
    l0 = np.ascontiguousarray((L0 * inv_sqrt_c).astype(np.float16))
    l1 = np.ascontiguousarray((L1 * inv_sqrt_c).astype(np.float16))
    idm = np.eye(C, dtype=np.float16)

    # --- species sort; equal per-core per-species segment lengths ---
    counts = np.bincount(sp, minlength=E)
    order = np.argsort(sp, kind="stable")
    starts = np.concatenate([[0], np.cumsum(counts)])
    q = [(int(counts[e]) + N_CORES - 1) // N_CORES for e in range(E)]
    n0 = sum(q)
    n_pad = ((n0 + 127) // 128) * 128
    q[E - 1] += n_pad - n0

    seg_bounds = []
    pos = 0
    for e in range(E):
        seg_bounds.append((pos, pos + q[e], e))
        pos += q[e]

    perms = []
    for k in range(N_CORES):
        parts = []
        for e in range(E):
            base_q = (int(counts[e]) + N_CORES - 1) // N_CORES
            lo = starts[e] + k * base_q
            hi = min(starts[e] + int(counts[e]), lo + base_q)
            seg = order[lo:hi] if hi > lo else order[starts[e]:starts[e] + 1]
            if len(seg) < q[e]:
                seg = np.concatenate([seg, np.repeat(seg[-1], q[e] - len(seg))])
            parts.append(seg)
        perms.append(np.concatenate(parts))

    sizes = _chunk_sizes(n_pad)
    in_maps = []
    for k in range(N_CORES):
        P = perms[k]
        # [n_pad, 128, 4] -> [4, 128, n_pad] planes (s, vx, vy, vz)
        nf_k = nf[P].transpose(2, 1, 0).astype(np.float16)
        sc_k = sc[P].transpose(2, 1, 0).astype(np.float16)
        nf_dev = np.empty((128, 4 * n_pad), np.float16)
        sc_dev = np.empty((128, 4 * n_pad), np.float16)
        b = 0
        for szc in sizes:
            blk = nf_k[:, :, b:b + szc].transpose(1, 0, 2).reshape(128, 4 * szc)
            nf_dev[:, 4 * b:4 * (b + szc)] = blk
            blk = sc_k[:, :, b:b + szc].transpose(1, 0, 2).reshape(128, 4 * szc)
            sc_dev[:, 4 * b:4 * (b + szc)] = blk
            b += szc
        in_maps.append({"nf": nf_dev, "sc": sc_dev, "wc": wc,
                        "l0": l0, "l1": l1, "idm": idm})
    return in_maps, seg_bounds, n_pad, perms


def _gather_output(res, n_pad, perms):
    sizes = _chunk_sizes(n_pad)
    y_full = np.empty((N_NODES, C, 4), np.float32)
    for k in range(N_CORES):
        o = res.results[k]["out"]
        y_sorted = np.empty((128, 4, n_pad), np.float32)
        b = 0
        for szc in sizes:
            blk = o[:, 4 * b:4 * (b + szc)].astype(np.float32)
            y_sorted[:, :, b:b + szc] = blk.reshape(128, 4, szc)
            b += szc
        # [d, plane, n] -> [n, d, plane]
        y_full[perms[k]] = y_sorted.transpose(2, 0, 1)
    return y_full


def kernel(**inputs):
    from concourse.bass_utils import run_bass_kernel_spmd

    in_maps, seg_bounds, n_pad, perms = _prep_host(inputs)
    key = (tuple(seg_bounds), n_pad)
    if _CACHE.get("key") != key:
        _CACHE["nc"] = _build_program(seg_bounds, n_pad)
        _CACHE["key"] = key
    nc = _CACHE["nc"]

    res = run_bass_kernel_spmd(nc, in_maps, core_ids=list(range(N_CORES)),
                               **_CACHE.get("run_kwargs", {}))
    _CACHE["last_result"] = res
    return _gather_output(res, n_pad, perms)
